# revision 1
# baseline (speedup 1.0000x reference)
"""Trainium2 Bass kernel for nn_CombinedTargetIOULoss (B=64, K=17, H=W=64).

Data-parallel over batch: 8 cores x 8 batches each. Each core computes
per-(b,k) partial sums [sum(q1+q2), sum((hp-hg)^2)] over the 4096 pixels;
the host combines them into the scalar loss (incl. target-weight scaling
and the tw==0 mask case).

Key algebra (the pixel anchors xs/ys cancel out of the reference box math):
  iw = (|p|+|g|-|p-g|)/2, cw = (|p|+|g|+|p-g|)/2  (same for y with q,h)
  inter = iw*ih, area_c = cw*ch, area_p = |p||q|, area_g = |g||h|
  union = area_p + area_g - inter + EPS
  giou_loss = 2 - inter/union - union/(area_c + EPS) = 2 - q1 - q2

SBUF layout: two batches stacked per tile, partition = (b%2)*64 + hx,
free = (ch=3k+c, hy). Every engine op covers all 128 partitions with a
uniform [128, (17,192),(64,1)] access pattern; per-(b,k) sums are done by
TensorE with one-hot stationary columns (psum row = local batch index).

Raw bass (no Tile): this walrus build rejects instructions carrying more
than one sem-wait, so all cross-engine sync is standalone wait_ge ops with
monotone per-engine counters.
"""

import sys

sys.path.insert(0, "/opt/trn_rl_repo")

import numpy as np

import concourse.bass as bass
from concourse import mybir
from concourse.alu_op_type import AluOpType as Alu
from concourse.bass_utils import run_bass_kernel_spmd
from concourse.dve_ops import (
    RECIP_APPROX_FAST_CONSTS as RAF_CONSTS,
    RECIPROCAL_APPROX_FAST as RAF_OP,
)

F32 = mybir.dt.float32
F16 = mybir.dt.float16
AF = mybir.ActivationFunctionType

EPS = 1e-7
B, K, H, W = 64, 17, 64, 64
C = 3 * K
P = H * W
N_CORES = 8
B_LOC = B // N_CORES
N_PAIR = B_LOC // 2

J = 64            # hy run (256B DMA descriptors)
MIDF = K * J      # 1088: free size of per-component intermediate tiles
INF = C * J       # 3264: free size of input tiles
# matmul column splits, k-aligned, each <= 512 cols and <= 1 PSUM bank
SPLITS = [(0, 6), (6, 6), (12, 5)]

N_DVE = 20        # DVE ops per pair-iteration
N_ACT = 8         # ACT ops per pair-iteration
N_PE = 6          # matmuls per pair-iteration


class _Waiter:
    """Dedupe monotone standalone waits per (engine, sem)."""

    def __init__(self):
        self.seen = {}

    def wait(self, eng, sem, val):
        key = (id(eng), sem.name if hasattr(sem, "name") else id(sem))
        if self.seen.get(key, -1) >= val:
            return
        self.seen[key] = val
        eng.wait_ge(sem, val)


def _build_body(nc, o_ext, t_ext, p_ext, repeat=1, mid_f16=False, gp_off=False,
                dma_cast16=False, dma_only=False, no_dma=False, tail_f16=False):
    MD = F16 if mid_f16 else F32
    IND = F16 if dma_cast16 else F32
    TD = F16 if tail_f16 else F32
    eps = 6.104e-5 if tail_f16 else EPS
    sb = lambda name, shape, dt: nc.alloc_sbuf_tensor(name, shape, dt).ap()

    # --- memory ---
    to = [sb(f"to{s}", [128, INF], IND) for s in range(2)]
    tt = [sb(f"tt{s}", [128, INF], IND) for s in range(2)]
    mids = {}
    for nm in "s ue rcu rcc ace".split():
        mids[nm] = sb(nm, [128, MIDF], TD)
    for nm in "ex ey d".split():
        mids[nm] = sb(nm, [128, MIDF], F16 if dma_cast16 else F32)
    for nm in "ap ag aq ah dx dy sx sy u2 uy2 v2 vy2 t1 t2 it4 ac4".split():
        mids[nm] = sb(nm, [128, MIDF], MD)
    for nm in ("q1", "q2"):
        mids[nm] = sb(nm, [128, MIDF], F16 if (mid_f16 or tail_f16) else F32)
    qs = sb("qs", [128, MIDF], F16)
    dsq = sb("dsq", [128, MIDF], F16)
    wts = [sb(f"w{j}", [128, B_LOC], F16) for j in range(N_PAIR)]
    osb = sb("osb", [B_LOC, 2 * K], F32)
    dmy = sb("dmy", [128, 4], F32)
    ps = []
    for qi in range(2):
        for si, (k0, n) in enumerate(SPLITS):
            ps.append(nc.alloc_psum_tensor(f"ps{qi}{si}", [B_LOC, n * J], F32).ap())

    # --- semaphores ---
    dma_in = nc.alloc_semaphore("dma_in")
    dma_out = nc.alloc_semaphore("dma_out")
    act_c = nc.alloc_semaphore("act_c")
    dve_c = nc.alloc_semaphore("dve_c")
    pe_c = nc.alloc_semaphore("pe_c")
    gp_c = nc.alloc_semaphore("gp_c")
    wt = _Waiter()

    # per-iteration op counts / in-iter positions
    ND = 17 if gp_off else 20          # DVE ops per iter
    DOF = 0 if gp_off else 3           # dve position offset of sx
    NG = 3 if gp_off else 0            # gpsimd ops per iter
    GP0 = 3 * N_PAIR                   # one-hot memsets precede loop

    def dpos(pos):                     # dve position of the box-algebra chain
        return DOF + pos

    # --- warmup: absorb ACT table loads on dependency-free instructions ---
    nc.scalar.activation(dmy[:, 0:1], dmy[:, 3:4], AF.Abs)
    nc.scalar.activation(dmy[:, 1:2], dmy[:, 3:4], AF.Square)
    nc.scalar.activation(dmy[:, 2:3], dmy[:, 3:4], AF.Copy, bias=0.0, scale=1.0)

    # --- one-hot stationary weights (GPSIMD) ---
    for j in range(N_PAIR):
        nc.gpsimd.memset(wts[j][:], 0.0).then_inc(gp_c, 1)
        nc.gpsimd.memset(wts[j][0:64, 2 * j : 2 * j + 1], 1.0).then_inc(gp_c, 1)
        nc.gpsimd.memset(wts[j][64:128, 2 * j + 1 : 2 * j + 2], 1.0).then_inc(gp_c, 1)

    def act(out, in_, func, **kw):
        nc.scalar.activation(out, in_, func, **kw).then_inc(act_c, 1)

    def dve_tt(out, a, b, op):
        nc.vector.tensor_tensor(out, a, b, op).then_inc(dve_c, 1)

    def comp(T, c):
        return T.rearrange("p (k c hy) -> p k c hy", k=K, c=3, hy=J)[:, :, c]

    m = lambda nm: mids[nm][:]

    n_iter = repeat * N_PAIR
    for j in range(n_iter):
        jp = j % N_PAIR       # which batch pair (repeat>1 reruns for timing)
        sl = j % 2
        dve0 = ND * j
        act0 = N_ACT * j
        gp0 = GP0 + NG * j

        # --- DMA in: WAR vs readers from iteration j-2 (same slot) ---
        dma_eng = nc.gpsimd if dma_cast16 else nc.sync
        if not no_dma:
            if j >= 2 and not dma_only:
                if gp_off:
                    wt.wait(dma_eng, gp_c, GP0 + NG * (j - 2) + 3)
                else:
                    wt.wait(dma_eng, dve_c, ND * (j - 2) + 3)
                wt.wait(dma_eng, act_c, N_ACT * (j - 2) + 4)
            for src, T in ((o_ext, to[sl]), (t_ext, tt[sl])):
                for pi in range(2):
                    dma_eng.dma_start(
                        out=T[64 * pi : 64 * pi + 64, :].rearrange(
                            "p (ch hy) -> p ch hy", ch=C, hy=J
                        ),
                        in_=src[2 * jp + pi].rearrange("ch hx hy -> hx ch hy"),
                    ).then_inc(dma_in, 16)
        if dma_only:
            continue

        # --- GPSIMD stream: raw-input diffs (optional offload) ---
        if gp_off:
            wt.wait(nc.gpsimd, dma_in, 64 * j + 64)
            if j >= 1:
                wt.wait(nc.gpsimd, act_c, N_ACT * (j - 1) + 7)  # dx,dy,dsq read
            nc.gpsimd.tensor_tensor(
                m("ex"), comp(to[sl], 1), comp(tt[sl], 1), Alu.subtract
            ).then_inc(gp_c, 1)
            nc.gpsimd.tensor_tensor(
                m("ey"), comp(to[sl], 2), comp(tt[sl], 2), Alu.subtract
            ).then_inc(gp_c, 1)
            nc.gpsimd.tensor_tensor(
                m("d"), comp(to[sl], 0), comp(tt[sl], 0), Alu.subtract
            ).then_inc(gp_c, 1)

        # --- ACT stream ---
        if j >= 1:
            wt.wait(nc.scalar, dve_c, ND * (j - 1) + dpos(8))   # t2 read ap..ah
        if not no_dma:
            wt.wait(nc.scalar, dma_in, 64 * j + 32)
        act(m("ap"), comp(to[sl], 1), AF.Abs)
        if not no_dma:
            wt.wait(nc.scalar, dma_in, 64 * j + 64)
        act(m("ag"), comp(tt[sl], 1), AF.Abs)
        act(m("aq"), comp(to[sl], 2), AF.Abs)
        act(m("ah"), comp(tt[sl], 2), AF.Abs)
        if gp_off:
            wt.wait(nc.scalar, gp_c, gp0 + 1)
            act(m("dx"), m("ex"), AF.Abs)
            wt.wait(nc.scalar, gp_c, gp0 + 2)
            act(m("dy"), m("ey"), AF.Abs)
            if j >= 1:
                wt.wait(nc.scalar, pe_c, N_PE * (j - 1) + 6)
            wt.wait(nc.scalar, gp_c, gp0 + 3)
            act(dsq[:], m("d"), AF.Square)
        else:
            wt.wait(nc.scalar, dve_c, dve0 + 1)
            act(m("dx"), m("ex"), AF.Abs)
            wt.wait(nc.scalar, dve_c, dve0 + 2)
            act(m("dy"), m("ey"), AF.Abs)
            if j >= 1:
                wt.wait(nc.scalar, pe_c, N_PE * (j - 1) + 6)
            wt.wait(nc.scalar, dve_c, dve0 + 3)
            act(dsq[:], m("d"), AF.Square)
        if j >= 1:
            wt.wait(nc.scalar, dve_c, ND * (j - 1) + dpos(14))  # rcc read ace
        wt.wait(nc.scalar, dve_c, dve0 + dpos(10))
        act(m("ace"), m("ac4"), AF.Copy, bias=eps, scale=0.25)

        # --- DVE stream ---
        if j >= 1:
            wt.wait(nc.vector, act_c, N_ACT * (j - 1) + 8)
        if not gp_off:
            if not no_dma:
                wt.wait(nc.vector, dma_in, 64 * j + 64)
            dve_tt(m("ex"), comp(to[sl], 1), comp(tt[sl], 1), Alu.subtract)
            dve_tt(m("ey"), comp(to[sl], 2), comp(tt[sl], 2), Alu.subtract)
            dve_tt(m("d"), comp(to[sl], 0), comp(tt[sl], 0), Alu.subtract)
        wt.wait(nc.vector, act_c, act0 + 2)
        dve_tt(m("sx"), m("ap"), m("ag"), Alu.add)                        # 1
        wt.wait(nc.vector, act_c, act0 + 4)
        dve_tt(m("sy"), m("aq"), m("ah"), Alu.add)                        # 2
        wt.wait(nc.vector, act_c, act0 + 5)
        dve_tt(m("u2"), m("sx"), m("dx"), Alu.subtract)                   # 3
        wt.wait(nc.vector, act_c, act0 + 6)
        dve_tt(m("uy2"), m("sy"), m("dy"), Alu.subtract)                  # 4
        dve_tt(m("v2"), m("sx"), m("dx"), Alu.add)                        # 5
        dve_tt(m("vy2"), m("sy"), m("dy"), Alu.add)                       # 6
        dve_tt(m("t1"), m("ap"), m("aq"), Alu.mult)                       # 7
        dve_tt(m("t2"), m("ag"), m("ah"), Alu.mult)                       # 8
        dve_tt(m("it4"), m("u2"), m("uy2"), Alu.mult)                     # 9
        dve_tt(m("ac4"), m("v2"), m("vy2"), Alu.mult)                     # 10
        nc.vector.scalar_tensor_tensor(
            m("s"), m("t1"), eps, m("t2"), Alu.add, Alu.add
        ).then_inc(dve_c, 1)                                              # 11
        nc.vector.scalar_tensor_tensor(
            m("ue"), m("it4"), -0.25, m("s"), Alu.mult, Alu.add
        ).then_inc(dve_c, 1)                                              # 12
        if tail_f16:
            _c = RAF_CONSTS
            nc.vector._custom_dve(RAF_OP, out=m("rcu"), in0=m("ue"),
                                  s0=_c["s0"], s1=_c["s1"], imm2=_c["imm2"]
                                  ).then_inc(dve_c, 1)                    # 13
        else:
            nc.vector.reciprocal_approx_fast(m("rcu"), m("ue")).then_inc(dve_c, 1)  # 13
        wt.wait(nc.vector, act_c, act0 + 8)
        if tail_f16:
            nc.vector._custom_dve(RAF_OP, out=m("rcc"), in0=m("ace"),
                                  s0=_c["s0"], s1=_c["s1"], imm2=_c["imm2"]
                                  ).then_inc(dve_c, 1)                    # 14
        else:
            nc.vector.reciprocal_approx_fast(m("rcc"), m("ace")).then_inc(dve_c, 1)  # 14
        nc.vector.scalar_tensor_tensor(
            m("q1"), m("it4"), 0.25, m("rcu"), Alu.mult, Alu.mult
        ).then_inc(dve_c, 1)                                              # 15
        dve_tt(m("q2"), m("ue"), m("rcc"), Alu.mult)                      # 16
        if j >= 1:
            wt.wait(nc.vector, pe_c, N_PE * (j - 1) + 3)
        dve_tt(qs[:], m("q1"), m("q2"), Alu.add)                          # 17

        # --- PE stream: per-(b,k) pixel sums ---
        if j == 0:
            wt.wait(nc.tensor, gp_c, GP0)
        for qi, qt in enumerate((qs, dsq)):
            if qi == 0:
                wt.wait(nc.tensor, dve_c, dve0 + ND)
            else:
                wt.wait(nc.tensor, act_c, act0 + 7)
            for si, (k0, n) in enumerate(SPLITS):
                nc.tensor.matmul(
                    ps[qi * 3 + si][:],
                    wts[jp][:],
                    qt[:, k0 * J : (k0 + n) * J],
                    start=(j == 0),
                    stop=(j == n_iter - 1),
                ).then_inc(pe_c, 1)

    # --- epilogue: reduce hy columns on DVE, then store ---
    if dma_only:
        wt.wait(nc.sync, dma_in, 64 * n_iter)
        nc.sync.dma_start(out=p_ext[:], in_=osb[:]).then_inc(dma_out, 16)
        nc.sync.wait_ge(dma_out, 16)
        return
    wt.wait(nc.vector, pe_c, N_PE * n_iter)
    nred = 0
    for qi in range(2):
        for si, (k0, n) in enumerate(SPLITS):
            pv = ps[qi * 3 + si].rearrange("p (k hy) -> p k hy", k=n, hy=J)
            nc.vector.tensor_reduce(
                osb[:, qi * K + k0 : qi * K + k0 + n],
                pv,
                mybir.AxisListType.X,
                Alu.add,
            ).then_inc(dve_c, 1)
            nred += 1
    wt.wait(nc.sync, dve_c, ND * n_iter + nred)
    nc.sync.dma_start(out=p_ext[:], in_=osb[:]).then_inc(dma_out, 16)
    nc.sync.wait_ge(dma_out, 16)


def build_nc(repeat=1, **kw):
    nc = bass.Bass()
    o_ext = nc.declare_dram_parameter("output", [B_LOC, C, H, W], F32, isOutput=False)
    t_ext = nc.declare_dram_parameter("target", [B_LOC, C, H, W], F32, isOutput=False)
    p_ext = nc.declare_dram_parameter("partials", [B_LOC, 2 * K], F32, isOutput=True)
    _build_body(nc, o_ext, t_ext, p_ext, repeat=repeat, **kw)
    # fill the 64-byte ISA encodings of custom DVE ops (reciprocal_approx):
    # Bacc.compile() does this; the raw-Bass + PJRT path does not.
    mybir.codegen_inst_isa_subclasses(nc)
    return nc


_NC = None


def _get_nc():
    global _NC
    if _NC is None:
        _NC = build_nc()
    return _NC


def _combine(parts, target_weights):
    """parts: [8 cores, 8, 34] f32 -> scalar loss (host-side finish)."""
    arr = np.asarray(parts, np.float64).reshape(B, 2 * K)
    sqs = arr[:, :K]        # sum over pixels of (q1 + q2), per (b, k)
    ssd = arr[:, K:]        # sum over pixels of (hp - hg)^2, per (b, k)

    tw = np.asarray(target_weights, np.float64)
    twnz = (tw != 0).astype(np.float64)
    num = ((2.0 * P - sqs) * twnz).sum(axis=0)
    den = np.maximum((P * twnz).sum(axis=0), 1.0)
    giou_joint = num / den
    mse = 0.5 * (tw**2 * ssd).sum(axis=0) / (B * P)
    return np.float32(np.sum(mse + giou_joint) / K)


def kernel(output, target, target_weights):
    output = np.ascontiguousarray(np.asarray(output), dtype=np.float32)
    target = np.ascontiguousarray(np.asarray(target), dtype=np.float32)
    nc = _get_nc()
    in_maps = [
        {
            "output": output[i * B_LOC : (i + 1) * B_LOC],
            "target": target[i * B_LOC : (i + 1) * B_LOC],
        }
        for i in range(N_CORES)
    ]
    res = run_bass_kernel_spmd(nc, in_maps, list(range(N_CORES)))
    parts = np.stack([res.results[i]["partials"] for i in range(N_CORES)])
    return np.asarray(_combine(parts, target_weights), dtype=np.float32)

